# revision 17
# baseline (speedup 1.0000x reference)
"""Trainium2 Bass kernel for an 8-layer ConditionalRealNVP flow.

Strategy
--------
Data-parallel: batch B=8192 is split across 8 NeuronCores (1024 rows each),
all parameters replicated. Activations live feature-major ([feature, batch])
in SBUF so every matmul is out[Mfeat, Nbatch] = W[K, M].T @ act[K, N] with
weights in natural [in, out] layout and outputs chaining directly into the
next matmul with no transposes.

All permutation / mask-split / scatter-merge index work is folded into the
weights on the host (input-independent):
  * z is kept in a fixed "storage order" on-device; a per-layer map
    storage-slot -> logical-feature is composed on the host.
  * x_a's contribution to the backbone input is W1z_eff = rows of W1[:A]
    scattered to the storage slots that hold x_a (other rows zero).
  * Ws/Wt columns are scattered to [H, 2D]: scale/shift for slot s land in
    columns s / D+s; x_a slots get zero columns (scale=0 -> exp=1, shift=0),
    so the affine update is uniformly  z <- z * exp(clip(scale)) + shift
    over all 256 slots, and logdet is a plain ones-vector matmul over the
    256 scale rows.
Matmul operands are bf16 (fp32 PSUM accumulation); z itself is carried in
fp32 across layers.
"""

import numpy as np
import ml_dtypes

import concourse.bass as bass
import concourse.mybir as mybir
import concourse.tile as tile
from concourse.bass_utils import run_bass_kernel_spmd

F32 = mybir.dt.float32
BF16 = mybir.dt.bfloat16
AF = mybir.ActivationFunctionType
ALU = mybir.AluOpType

L, D, H, B = 8, 256, 1024, 8192
CLIP = 3.0
A = D // 2
N_CORES = 8
P = 128
KC = 3 * H // P      # 24 ctx k-subtiles
KZ = D // P          # 2 z k-subtiles (emitted last in the mm1 K loop)
K1 = KC + KZ         # 26
MH = H // P          # 8 output tiles of the hidden layers
MST = 2 * D // P     # 4 output tiles of the scale/shift head
bf16 = ml_dtypes.bfloat16


def _prep(perm, idx_a, idx_b, inv_pos, W1, Ws, bs, Wt, bt):
    """Fold all index plumbing into weights; returns fp32 arrays + final map."""
    m = np.arange(D)
    W1z = np.zeros((L, D, H), np.float32)
    Wst = np.zeros((L, H, 2 * D), np.float32)
    bst = np.zeros((L, 2 * D), np.float32)
    for l in range(L):
        pos = np.empty(D, np.int64)
        pos[m] = np.arange(D)
        slots_a = pos[perm[l][idx_a[l]]]
        slots_b = pos[perm[l][idx_b[l]]]
        W1z[l][slots_a, :] = W1[l][:A, :]
        Wst[l][:, slots_b] = Ws[l]
        Wst[l][:, D + slots_b] = Wt[l]
        bst[l][slots_b] = bs[l]
        bst[l][D + slots_b] = bt[l]
        m_new = np.empty(D, np.int64)
        i = np.asarray(inv_pos[l])
        m_new[np.where(i < A, slots_a[np.minimum(i, A - 1)],
                       slots_b[np.maximum(i - A, 0)])] = np.arange(D)
        m = m_new
    return W1z, Wst, bst, m


def _wtiles(w, km, mm):
    """[K*P? , M*P] fp32 -> [mm, P, km, P] bf16 tiles, partition-contiguous."""
    k, mdim = w.shape
    assert k == km * P and mdim == mm * P
    return np.ascontiguousarray(
        w.reshape(km, P, mm, P).transpose(2, 1, 0, 3)
    ).astype(bf16)


_NO_SPLIT = ("InstAllEngineBarrier", "InstCollectiveCompute",
             "InstEventSemaphore")


def _legalize_waits(nc, max_waits=1):
    """walrus codegen allows one sync-wait command per compute instruction;
    move extra waits onto preceding same-engine EventSemaphore carriers."""
    import concourse.mybir as mb
    ctr = 0
    for f in nc.m.functions:
        for b in f.blocks:
            il = b.instructions
            out = []
            changed = False
            for ins in il:
                si = getattr(ins, "sync_info", None)
                waits = list(si.on_wait) if si is not None and si.on_wait else []
                if len(waits) > max_waits and type(ins).__name__ not in _NO_SPLIT:
                    for w in waits[:-max_waits]:
                        ev = mb.InstEventSemaphore(
                            name=f"evsplit-{ctr}", ins=[], outs=[])
                        ctr += 1
                        ev.engine = ins.engine
                        ev.sync_info = mb.SyncInfo(on_wait=[w], on_update=[])
                        out.append(ev)
                    ins.sync_info = mb.SyncInfo(
                        on_wait=waits[-max_waits:], on_update=list(si.on_update))
                    changed = True
                out.append(ins)
            if changed:
                b.instructions = out
    return nc


def build_nc(NB, legalize=True):
    """Emit the Bass/Tile program for a per-core batch of NB rows."""
    NH = NB // 2
    nc = bass.Bass("TRN2", target_bir_lowering=False, debug=False,
                   num_devices=N_CORES)

    zt_d = nc.dram_tensor("zt", [P, KZ, NB], F32, kind="ExternalInput").ap()
    ct_d = nc.dram_tensor("ct", [KC, P, NB], BF16, kind="ExternalInput").ap()
    w1_d = nc.dram_tensor("w1", [L, MH, P, K1, P], BF16, kind="ExternalInput").ap()
    w2_d = nc.dram_tensor("w2", [L, MH, P, MH, P], BF16, kind="ExternalInput").ap()
    w3_d = nc.dram_tensor("w3", [L, MH, P, MH, P], BF16, kind="ExternalInput").ap()
    wst_d = nc.dram_tensor("wst", [L, MST, P, MH, P], BF16, kind="ExternalInput").ap()
    b123_d = nc.dram_tensor("b123", [P, 3, L, MH], F32, kind="ExternalInput").ap()
    bst_d = nc.dram_tensor("bst", [P, L, MST], F32, kind="ExternalInput").ap()
    oz_d = nc.dram_tensor("oz", [P, KZ, NB], F32, kind="ExternalOutput").ap()
    old_d = nc.dram_tensor("old", [1, NB], F32, kind="ExternalOutput").ap()

    with tile.TileContext(nc) as tc:
        with (
            tc.tile_pool(name="singles", bufs=1) as singles,
            tc.tile_pool(name="wpool", bufs=4) as wpool,
            tc.tile_pool(name="psum", bufs=4, space="PSUM") as psum,
            tc.tile_pool(name="ldps", bufs=2, space="PSUM") as ldps,
        ):
            # --- resident tensors ---
            ct_sb = singles.tile([P, KC, NB], BF16, tag="ct")
            for k in range(KC):
                nc.sync.dma_start(out=ct_sb[:, k, :], in_=ct_d[k])
            z32 = singles.tile([P, KZ, NB], F32, tag="z32")
            nc.sync.dma_start(out=z32[:], in_=zt_d[:])
            zbf = singles.tile([P, KZ, NB], BF16, tag="zbf")
            nc.vector.tensor_copy(out=zbf[:], in_=z32[:])

            b123 = singles.tile([P, 3, L, MH], F32, tag="b123")
            nc.sync.dma_start(out=b123[:], in_=b123_d[:])
            bst_sb = singles.tile([P, L, MST], F32, tag="bst")
            nc.sync.dma_start(out=bst_sb[:], in_=bst_d[:])

            ones_sb = singles.tile([P, 1], BF16, tag="ones")
            nc.vector.memset(ones_sb, 1.0)
            ld_acc = singles.tile([1, NB], F32, tag="ldacc")
            nc.vector.memset(ld_acc, 0.0)

            h_sb = [singles.tile([P, MH, NB], BF16, tag=f"h{i}", name=f"h{i}")
                    for i in range(3)]
            st_tmp = singles.tile([P, 2, NB], F32, tag="sttmp")
            sc = singles.tile([P, 2, NB], BF16, tag="sc")
            ex = singles.tile([P, 2, NB], F32, tag="ex")
            sh = singles.tile([P, 2, NB], F32, tag="sh")

            def mm_layer(w_dram, l, km, mm, rhs_of_k, consume):
                """out[m] = W[:, m].T @ act  for m in range(mm), K = km tiles."""
                for m in range(mm):
                    wt = wpool.tile([P, km, P], BF16, tag=f"w{km}")
                    nc.sync.dma_start(out=wt[:], in_=w_dram[l, m])
                    ps = [psum.tile([P, NH], F32, tag="ps", name=f"ps{i}")
                          for i in range(2)]
                    for k in range(km):
                        rhs = rhs_of_k(k)
                        for nh in range(2):
                            nc.tensor.matmul(
                                ps[nh], wt[:, k], rhs[:, nh * NH:(nh + 1) * NH],
                                start=(k == 0), stop=(k == km - 1),
                            )
                    for nh in range(2):
                        consume(m, nh, ps[nh])

            for l in range(L):
                def silu_to(dst, bvec):
                    def c(m, nh, ps):
                        nc.scalar.activation(
                            out=dst[:, m, nh * NH:(nh + 1) * NH], in_=ps,
                            func=AF.Silu, bias=bvec[:, l, m:m + 1], scale=1.0,
                        )
                    return c

                # mm1: backbone in = [ctx3 ; z]  (z tiles last so the PE can
                # start each layer before the previous z-update finishes)
                mm_layer(
                    w1_d, l, K1, MH,
                    lambda k: ct_sb[:, k] if k < KC else zbf[:, k - KC],
                    silu_to(h_sb[0], b123[:, 0]),
                )
                mm_layer(w2_d, l, MH, MH, lambda k: h_sb[0][:, k],
                         silu_to(h_sb[1], b123[:, 1]))
                mm_layer(w3_d, l, MH, MH, lambda k: h_sb[1][:, k],
                         silu_to(h_sb[2], b123[:, 2]))

                # mm4: scale/shift head -> st_tmp (pre-clip scale) and sh
                def st_consume(m, nh, ps):
                    dst = st_tmp[:, m] if m < 2 else sh[:, m - 2]
                    nc.scalar.activation(
                        out=dst[:, nh * NH:(nh + 1) * NH], in_=ps,
                        func=AF.Identity, bias=bst_sb[:, l, m:m + 1], scale=1.0,
                    )
                mm_layer(wst_d, l, MH, MST, lambda k: h_sb[2][:, k], st_consume)

                # sc = clip(st_tmp) (bf16), ex = exp(sc)
                nc.vector.tensor_scalar(
                    out=sc[:], in0=st_tmp[:], scalar1=CLIP, scalar2=-CLIP,
                    op0=ALU.min, op1=ALU.max,
                )
                nc.scalar.activation(out=ex[:], in_=sc[:], func=AF.Exp)

                # logdet += ones.T @ sc  (sum over the 256 scale rows)
                for nh in range(2):
                    ldp = ldps.tile([1, NH], F32, tag="ld")
                    for k in range(KZ):
                        nc.tensor.matmul(
                            ldp, ones_sb, sc[:, k, nh * NH:(nh + 1) * NH],
                            start=(k == 0), stop=(k == KZ - 1),
                        )
                    nc.vector.tensor_add(
                        out=ld_acc[:, nh * NH:(nh + 1) * NH],
                        in0=ld_acc[:, nh * NH:(nh + 1) * NH], in1=ldp,
                    )

                # z <- z * ex + sh ; refresh bf16 copy for next layer's mm1
                nc.vector.tensor_mul(out=z32[:], in0=z32[:], in1=ex[:])
                nc.vector.tensor_add(out=z32[:], in0=z32[:], in1=sh[:])
                nc.vector.tensor_copy(out=zbf[:], in_=z32[:])

            nc.sync.dma_start(out=oz_d[:], in_=z32[:])
            nc.sync.dma_start(out=old_d[:], in_=ld_acc[:])
    return _legalize_waits(nc) if legalize else nc


def host_prep(inputs):
    """All input-independent restructuring. Returns (weight in_map, m_L)."""
    W1z, Wst, bst, m_L = _prep(
        np.asarray(inputs["perm"]), np.asarray(inputs["idx_a"]),
        np.asarray(inputs["idx_b"]), np.asarray(inputs["inv_pos"]),
        np.asarray(inputs["W1"], np.float32), np.asarray(inputs["Ws"], np.float32),
        np.asarray(inputs["bs"], np.float32), np.asarray(inputs["Wt"], np.float32),
        np.asarray(inputs["bt"], np.float32))
    W1 = np.asarray(inputs["W1"], np.float32)
    w1 = np.stack([
        _wtiles(np.concatenate([W1[l][A:], W1z[l]], axis=0), K1, MH)
        for l in range(L)
    ])
    w2 = np.stack([_wtiles(np.asarray(inputs["W2"][l], np.float32), MH, MH)
                   for l in range(L)])
    w3 = np.stack([_wtiles(np.asarray(inputs["W3"][l], np.float32), MH, MH)
                   for l in range(L)])
    wst = np.stack([_wtiles(Wst[l], MH, MST) for l in range(L)])
    b123 = np.ascontiguousarray(
        np.stack([np.asarray(inputs[n], np.float32) for n in ("b1", "b2", "b3")])
        .reshape(3, L, MH, P).transpose(3, 0, 1, 2))
    bsth = np.ascontiguousarray(bst.reshape(L, MST, P).transpose(2, 0, 1))
    wm = {"w1": w1, "w2": w2, "w3": w3, "wst": wst, "b123": b123, "bst": bsth}
    return wm, m_L


def core_inputs(inputs, NB=B // N_CORES):
    """Per-core activation shards (feature-major)."""
    x = np.asarray(inputs["x"], np.float32)
    c3 = np.concatenate([np.asarray(inputs["ctx"], np.float32),
                         np.asarray(inputs["t_emb"], np.float32),
                         np.asarray(inputs["cond_emb"], np.float32)], axis=1)
    outs = []
    for c in range(N_CORES):
        rows = slice(c * NB, (c + 1) * NB)
        zt = np.ascontiguousarray(
            x[rows].T.reshape(KZ, P, NB).transpose(1, 0, 2))
        ct = np.ascontiguousarray(c3[rows].T.astype(bf16)).reshape(KC, P, NB)
        outs.append({"zt": zt, "ct": ct})
    return outs


_NC_CACHE = {}


def kernel(**inputs):
    NB = B // N_CORES
    if NB not in _NC_CACHE:
        _NC_CACHE[NB] = build_nc(NB)
    nc = _NC_CACHE[NB]
    wm, m_L = host_prep(inputs)
    shards = core_inputs(inputs, NB)
    in_maps = [{**s, **wm} for s in shards]
    res = run_bass_kernel_spmd(nc, in_maps, list(range(N_CORES)))
    z_store = np.concatenate(
        [res.results[c]["oz"].transpose(1, 0, 2).reshape(D, NB).T
         for c in range(N_CORES)], axis=0)
    z = np.ascontiguousarray(z_store[:, np.argsort(m_L)], dtype=np.float32)
    logdet = np.concatenate(
        [res.results[c]["old"][0] for c in range(N_CORES)]).astype(np.float32)
    return z, logdet


# revision 21
# speedup vs baseline: 1.2034x; 1.2034x over previous
"""Trainium2 Bass kernel for an 8-layer ConditionalRealNVP flow.

Strategy
--------
Data-parallel: batch B=8192 is split across 8 NeuronCores (1024 rows each),
all parameters replicated. Activations live feature-major ([feature, batch])
in SBUF so every matmul is out[Mfeat, Nbatch] = W[K, M].T @ act[K, N] with
weights in natural [in, out] layout and outputs chaining directly into the
next matmul with no transposes.

All permutation / mask-split / scatter-merge index work is folded into the
weights on the host (input-independent):
  * z is kept in a fixed "storage order" on-device; a per-layer map
    storage-slot -> logical-feature is composed on the host.
  * x_a's contribution to the backbone input is W1z_eff = rows of W1[:A]
    scattered to the storage slots that hold x_a (other rows zero).
  * Ws/Wt columns are scattered to [H, 2D]: scale/shift for slot s land in
    columns s / D+s; x_a slots get zero columns (scale=0 -> exp=1, shift=0),
    so the affine update is uniformly  z <- z * exp(clip(scale)) + shift
    over all 256 slots, and logdet is a plain ones-vector matmul over the
    256 scale rows.
Matmul operands are bf16 (fp32 PSUM accumulation); z itself is carried in
fp32 across layers.
"""

import numpy as np
import ml_dtypes

import concourse.bass as bass
import concourse.mybir as mybir
import concourse.tile as tile
from concourse.bass_utils import run_bass_kernel_spmd

F32 = mybir.dt.float32
BF16 = mybir.dt.bfloat16
AF = mybir.ActivationFunctionType
ALU = mybir.AluOpType

L, D, H, B = 8, 256, 1024, 8192
CLIP = 3.0
A = D // 2
N_CORES = 8
P = 128
KC = 3 * H // P      # 24 ctx k-subtiles
KZ = D // P          # 2 z k-subtiles (emitted last in the mm1 K loop)
K1 = KC + KZ         # 26
MH = H // P          # 8 output tiles of the hidden layers
MST = 2 * D // P     # 4 output tiles of the scale/shift head
bf16 = ml_dtypes.bfloat16


def _prep(perm, idx_a, idx_b, inv_pos, W1, Ws, bs, Wt, bt):
    """Fold all index plumbing into weights; returns fp32 arrays + final map."""
    m = np.arange(D)
    W1z = np.zeros((L, D, H), np.float32)
    Wst = np.zeros((L, H, 2 * D), np.float32)
    bst = np.zeros((L, 2 * D), np.float32)
    for l in range(L):
        pos = np.empty(D, np.int64)
        pos[m] = np.arange(D)
        slots_a = pos[perm[l][idx_a[l]]]
        slots_b = pos[perm[l][idx_b[l]]]
        W1z[l][slots_a, :] = W1[l][:A, :]
        Wst[l][:, slots_b] = Ws[l]
        Wst[l][:, D + slots_b] = Wt[l]
        bst[l][slots_b] = bs[l]
        bst[l][D + slots_b] = bt[l]
        m_new = np.empty(D, np.int64)
        i = np.asarray(inv_pos[l])
        m_new[np.where(i < A, slots_a[np.minimum(i, A - 1)],
                       slots_b[np.maximum(i - A, 0)])] = np.arange(D)
        m = m_new
    return W1z, Wst, bst, m


def _wtiles(w, km, mm):
    """[K*P? , M*P] fp32 -> [mm, P, km, P] bf16 tiles, partition-contiguous."""
    k, mdim = w.shape
    assert k == km * P and mdim == mm * P
    return np.ascontiguousarray(
        w.reshape(km, P, mm, P).transpose(2, 1, 0, 3)
    ).astype(bf16)


_NO_SPLIT = ("InstAllEngineBarrier", "InstCollectiveCompute",
             "InstEventSemaphore")


def _dedup_ldweights(nc):
    """Tile legalization emits one LDWEIGHTS per matmul; consecutive matmuls
    over the two batch halves share lhsT, so every second load is redundant
    (~108 ns x 2960 here). Drop an LDWEIGHTS when the previous PE instruction
    stream since the last LDWEIGHTS contains only matmuls and the weight AP
    is identical. Any waits it carried move to an EventSemaphore."""
    import concourse.mybir as mb
    ctr = 0
    for f in nc.m.functions:
        for b in f.blocks:
            out = []
            last_sig = None
            changed = False
            for ins in b.instructions:
                tn = type(ins).__name__
                if tn == "InstLdweights":
                    w = ins.ins[0]
                    sig = (str(w.memref), w.offset, str(w.ap), str(w.dtype),
                           str(ins.perf_mode), str(ins.is_transpose),
                           str(ins.tile_position))
                    if sig == last_sig:
                        si = getattr(ins, "sync_info", None)
                        waits = list(si.on_wait) if si is not None and si.on_wait else []
                        upds = list(si.on_update) if si is not None and si.on_update else []
                        for w_ in waits:
                            ev = mb.InstEventSemaphore(
                                name=f"ldwev-{ctr}", ins=[], outs=[])
                            ctr += 1
                            ev.engine = ins.engine
                            ev.sync_info = mb.SyncInfo(on_wait=[w_], on_update=[])
                            out.append(ev)
                        assert not upds, "dropped LDWEIGHTS carried an update"
                        changed = True
                        continue  # drop the redundant load
                    last_sig = sig
                elif getattr(ins, "engine", None) == mb.EngineType.PE \
                        and tn not in ("InstMatmult",):
                    last_sig = None  # anything else on PE invalidates reuse
                out.append(ins)
            if changed:
                b.instructions = out
    return nc


def _legalize_waits(nc, max_waits=1):
    """walrus codegen allows one sync-wait command per compute instruction;
    move extra waits onto preceding same-engine EventSemaphore carriers."""
    import concourse.mybir as mb
    ctr = 0
    for f in nc.m.functions:
        for b in f.blocks:
            il = b.instructions
            out = []
            changed = False
            for ins in il:
                si = getattr(ins, "sync_info", None)
                waits = list(si.on_wait) if si is not None and si.on_wait else []
                if len(waits) > max_waits and type(ins).__name__ not in _NO_SPLIT:
                    for w in waits[:-max_waits]:
                        ev = mb.InstEventSemaphore(
                            name=f"evsplit-{ctr}", ins=[], outs=[])
                        ctr += 1
                        ev.engine = ins.engine
                        ev.sync_info = mb.SyncInfo(on_wait=[w], on_update=[])
                        out.append(ev)
                    ins.sync_info = mb.SyncInfo(
                        on_wait=waits[-max_waits:], on_update=list(si.on_update))
                    changed = True
                out.append(ins)
            if changed:
                b.instructions = out
    return nc


def build_nc(NB, legalize=True):
    """Emit the Bass/Tile program for a per-core batch of NB rows."""
    NH = NB // 2
    nc = bass.Bass("TRN2", target_bir_lowering=False, debug=False,
                   num_devices=N_CORES)

    zt_d = nc.dram_tensor("zt", [P, KZ, NB], F32, kind="ExternalInput").ap()
    ct_d = nc.dram_tensor("ct", [KC, P, NB], BF16, kind="ExternalInput").ap()
    w1_d = nc.dram_tensor("w1", [L, MH, P, K1, P], BF16, kind="ExternalInput").ap()
    w2_d = nc.dram_tensor("w2", [L, MH, P, MH, P], BF16, kind="ExternalInput").ap()
    w3_d = nc.dram_tensor("w3", [L, MH, P, MH, P], BF16, kind="ExternalInput").ap()
    wst_d = nc.dram_tensor("wst", [L, MST, P, MH, P], BF16, kind="ExternalInput").ap()
    b123_d = nc.dram_tensor("b123", [P, 3, L, MH], F32, kind="ExternalInput").ap()
    bst_d = nc.dram_tensor("bst", [P, L, MST], F32, kind="ExternalInput").ap()
    oz_d = nc.dram_tensor("oz", [P, KZ, NB], F32, kind="ExternalOutput").ap()
    old_d = nc.dram_tensor("old", [1, NB], F32, kind="ExternalOutput").ap()

    with tile.TileContext(nc) as tc:
        with (
            tc.tile_pool(name="singles", bufs=1) as singles,
            tc.tile_pool(name="wpool", bufs=4) as wpool,
            tc.tile_pool(name="psum", bufs=4, space="PSUM") as psum,
            tc.tile_pool(name="ldps", bufs=2, space="PSUM") as ldps,
        ):
            # --- resident tensors ---
            ct_sb = singles.tile([P, KC, NB], BF16, tag="ct")
            for k in range(KC):
                nc.sync.dma_start(out=ct_sb[:, k, :], in_=ct_d[k])
            z32 = singles.tile([P, KZ, NB], F32, tag="z32")
            nc.sync.dma_start(out=z32[:], in_=zt_d[:])
            zbf = singles.tile([P, KZ, NB], BF16, tag="zbf")
            nc.vector.tensor_copy(out=zbf[:], in_=z32[:])

            b123 = singles.tile([P, 3, L, MH], F32, tag="b123")
            nc.sync.dma_start(out=b123[:], in_=b123_d[:])
            bst_sb = singles.tile([P, L, MST], F32, tag="bst")
            nc.sync.dma_start(out=bst_sb[:], in_=bst_d[:])

            ones_sb = singles.tile([P, 1], BF16, tag="ones")
            nc.vector.memset(ones_sb, 1.0)
            ld_acc = singles.tile([1, NB], F32, tag="ldacc")
            nc.vector.memset(ld_acc, 0.0)

            h_sb = [singles.tile([P, MH, NB], BF16, tag=f"h{i}", name=f"h{i}")
                    for i in range(3)]
            st_tmp = singles.tile([P, 2, NB], F32, tag="sttmp")
            sc = singles.tile([P, 2, NB], BF16, tag="sc")
            ex = singles.tile([P, 2, NB], F32, tag="ex")
            sh = singles.tile([P, 2, NB], F32, tag="sh")

            def mm_layer(w_dram, l, km, mm, rhs_of_k, consume):
                """out[m] = W[:, m].T @ act  for m in range(mm), K = km tiles."""
                for m in range(mm):
                    wt = wpool.tile([P, km, P], BF16, tag=f"w{km}")
                    nc.sync.dma_start(out=wt[:], in_=w_dram[l, m])
                    ps = [psum.tile([P, NH], F32, tag="ps", name=f"ps{i}")
                          for i in range(2)]
                    for k in range(km):
                        rhs = rhs_of_k(k)
                        for nh in range(2):
                            nc.tensor.matmul(
                                ps[nh], wt[:, k], rhs[:, nh * NH:(nh + 1) * NH],
                                start=(k == 0), stop=(k == km - 1),
                            )
                    for nh in range(2):
                        consume(m, nh, ps[nh])

            for l in range(L):
                def silu_to(dst, bvec):
                    def c(m, nh, ps):
                        nc.scalar.activation(
                            out=dst[:, m, nh * NH:(nh + 1) * NH], in_=ps,
                            func=AF.Silu, bias=bvec[:, l, m:m + 1], scale=1.0,
                        )
                    return c

                # mm1: backbone in = [ctx3 ; z]  (z tiles last so the PE can
                # start each layer before the previous z-update finishes)
                mm_layer(
                    w1_d, l, K1, MH,
                    lambda k: ct_sb[:, k] if k < KC else zbf[:, k - KC],
                    silu_to(h_sb[0], b123[:, 0]),
                )
                mm_layer(w2_d, l, MH, MH, lambda k: h_sb[0][:, k],
                         silu_to(h_sb[1], b123[:, 1]))
                mm_layer(w3_d, l, MH, MH, lambda k: h_sb[1][:, k],
                         silu_to(h_sb[2], b123[:, 2]))

                # mm4: scale/shift head -> st_tmp (pre-clip scale) and sh
                def st_consume(m, nh, ps):
                    dst = st_tmp[:, m] if m < 2 else sh[:, m - 2]
                    nc.scalar.activation(
                        out=dst[:, nh * NH:(nh + 1) * NH], in_=ps,
                        func=AF.Identity, bias=bst_sb[:, l, m:m + 1], scale=1.0,
                    )
                mm_layer(wst_d, l, MH, MST, lambda k: h_sb[2][:, k], st_consume)

                # sc = clip(st_tmp) (bf16), ex = exp(sc)
                nc.vector.tensor_scalar(
                    out=sc[:], in0=st_tmp[:], scalar1=CLIP, scalar2=-CLIP,
                    op0=ALU.min, op1=ALU.max,
                )
                nc.scalar.activation(out=ex[:], in_=sc[:], func=AF.Exp)

                # logdet += ones.T @ sc  (sum over the 256 scale rows)
                for nh in range(2):
                    ldp = ldps.tile([1, NH], F32, tag="ld")
                    for k in range(KZ):
                        nc.tensor.matmul(
                            ldp, ones_sb, sc[:, k, nh * NH:(nh + 1) * NH],
                            start=(k == 0), stop=(k == KZ - 1),
                        )
                    nc.vector.tensor_add(
                        out=ld_acc[:, nh * NH:(nh + 1) * NH],
                        in0=ld_acc[:, nh * NH:(nh + 1) * NH], in1=ldp,
                    )

                # z <- z * ex + sh ; refresh bf16 copy for next layer's mm1
                nc.vector.tensor_mul(out=z32[:], in0=z32[:], in1=ex[:])
                nc.vector.tensor_add(out=z32[:], in0=z32[:], in1=sh[:])
                nc.vector.tensor_copy(out=zbf[:], in_=z32[:])

            nc.sync.dma_start(out=oz_d[:], in_=z32[:])
            nc.sync.dma_start(out=old_d[:], in_=ld_acc[:])
    return _legalize_waits(_dedup_ldweights(nc)) if legalize else nc


def host_prep(inputs):
    """All input-independent restructuring. Returns (weight in_map, m_L)."""
    W1z, Wst, bst, m_L = _prep(
        np.asarray(inputs["perm"]), np.asarray(inputs["idx_a"]),
        np.asarray(inputs["idx_b"]), np.asarray(inputs["inv_pos"]),
        np.asarray(inputs["W1"], np.float32), np.asarray(inputs["Ws"], np.float32),
        np.asarray(inputs["bs"], np.float32), np.asarray(inputs["Wt"], np.float32),
        np.asarray(inputs["bt"], np.float32))
    W1 = np.asarray(inputs["W1"], np.float32)
    w1 = np.stack([
        _wtiles(np.concatenate([W1[l][A:], W1z[l]], axis=0), K1, MH)
        for l in range(L)
    ])
    w2 = np.stack([_wtiles(np.asarray(inputs["W2"][l], np.float32), MH, MH)
                   for l in range(L)])
    w3 = np.stack([_wtiles(np.asarray(inputs["W3"][l], np.float32), MH, MH)
                   for l in range(L)])
    wst = np.stack([_wtiles(Wst[l], MH, MST) for l in range(L)])
    b123 = np.ascontiguousarray(
        np.stack([np.asarray(inputs[n], np.float32) for n in ("b1", "b2", "b3")])
        .reshape(3, L, MH, P).transpose(3, 0, 1, 2))
    bsth = np.ascontiguousarray(bst.reshape(L, MST, P).transpose(2, 0, 1))
    wm = {"w1": w1, "w2": w2, "w3": w3, "wst": wst, "b123": b123, "bst": bsth}
    return wm, m_L


def core_inputs(inputs, NB=B // N_CORES):
    """Per-core activation shards (feature-major)."""
    x = np.asarray(inputs["x"], np.float32)
    c3 = np.concatenate([np.asarray(inputs["ctx"], np.float32),
                         np.asarray(inputs["t_emb"], np.float32),
                         np.asarray(inputs["cond_emb"], np.float32)], axis=1)
    outs = []
    for c in range(N_CORES):
        rows = slice(c * NB, (c + 1) * NB)
        zt = np.ascontiguousarray(
            x[rows].T.reshape(KZ, P, NB).transpose(1, 0, 2))
        ct = np.ascontiguousarray(c3[rows].T.astype(bf16)).reshape(KC, P, NB)
        outs.append({"zt": zt, "ct": ct})
    return outs


_NC_CACHE = {}


def kernel(**inputs):
    NB = B // N_CORES
    if NB not in _NC_CACHE:
        _NC_CACHE[NB] = build_nc(NB)
    nc = _NC_CACHE[NB]
    wm, m_L = host_prep(inputs)
    shards = core_inputs(inputs, NB)
    in_maps = [{**s, **wm} for s in shards]
    res = run_bass_kernel_spmd(nc, in_maps, list(range(N_CORES)))
    z_store = np.concatenate(
        [res.results[c]["oz"].transpose(1, 0, 2).reshape(D, NB).T
         for c in range(N_CORES)], axis=0)
    z = np.ascontiguousarray(z_store[:, np.argsort(m_L)], dtype=np.float32)
    logdet = np.concatenate(
        [res.results[c]["old"][0] for c in range(N_CORES)]).astype(np.float32)
    return z, logdet


# revision 25
# speedup vs baseline: 1.2153x; 1.0099x over previous
"""Trainium2 Bass kernel for an 8-layer ConditionalRealNVP flow.

Strategy
--------
Data-parallel: batch B=8192 is split across 8 NeuronCores (1024 rows each),
all parameters replicated. Activations live feature-major ([feature, batch])
in SBUF so every matmul is out[Mfeat, Nbatch] = W[K, M].T @ act[K, N] with
weights in natural [in, out] layout and outputs chaining directly into the
next matmul with no transposes.

All permutation / mask-split / scatter-merge index work is folded into the
weights on the host (input-independent):
  * z is kept in a fixed "storage order" on-device; a per-layer map
    storage-slot -> logical-feature is composed on the host.
  * x_a's contribution to the backbone input is W1z_eff = rows of W1[:A]
    scattered to the storage slots that hold x_a (other rows zero).
  * Ws/Wt columns are scattered to [H, 2D]: scale/shift for slot s land in
    columns s / D+s; x_a slots get zero columns (scale=0 -> exp=1, shift=0),
    so the affine update is uniformly  z <- z * exp(clip(scale)) + shift
    over all 256 slots, and logdet is a plain ones-vector matmul over the
    256 scale rows.
Matmul operands are bf16 (fp32 PSUM accumulation); z itself is carried in
fp32 across layers.
"""

import numpy as np
import ml_dtypes

import concourse.bass as bass
import concourse.mybir as mybir
import concourse.tile as tile
from concourse.bass_utils import run_bass_kernel_spmd

F32 = mybir.dt.float32
BF16 = mybir.dt.bfloat16
AF = mybir.ActivationFunctionType
ALU = mybir.AluOpType

L, D, H, B = 8, 256, 1024, 8192
CLIP = 3.0
A = D // 2
N_CORES = 8
P = 128
KC = 3 * H // P      # 24 ctx k-subtiles
KZ = D // P          # 2 z k-subtiles (emitted last in the mm1 K loop)
K1 = KC + KZ         # 26
MH = H // P          # 8 output tiles of the hidden layers
MST = 2 * D // P     # 4 output tiles of the scale/shift head
bf16 = ml_dtypes.bfloat16


def _prep(perm, idx_a, idx_b, inv_pos, W1, Ws, bs, Wt, bt):
    """Fold all index plumbing into weights; returns fp32 arrays + final map."""
    m = np.arange(D)
    W1z = np.zeros((L, D, H), np.float32)
    Wst = np.zeros((L, H, 2 * D), np.float32)
    bst = np.zeros((L, 2 * D), np.float32)
    for l in range(L):
        pos = np.empty(D, np.int64)
        pos[m] = np.arange(D)
        slots_a = pos[perm[l][idx_a[l]]]
        slots_b = pos[perm[l][idx_b[l]]]
        W1z[l][slots_a, :] = W1[l][:A, :]
        Wst[l][:, slots_b] = Ws[l]
        Wst[l][:, D + slots_b] = Wt[l]
        bst[l][slots_b] = bs[l]
        bst[l][D + slots_b] = bt[l]
        m_new = np.empty(D, np.int64)
        i = np.asarray(inv_pos[l])
        m_new[np.where(i < A, slots_a[np.minimum(i, A - 1)],
                       slots_b[np.maximum(i - A, 0)])] = np.arange(D)
        m = m_new
    return W1z, Wst, bst, m


def _wtiles(w, km, mm):
    """[K*P? , M*P] fp32 -> [mm, P, km, P] bf16 tiles, partition-contiguous."""
    k, mdim = w.shape
    assert k == km * P and mdim == mm * P
    return np.ascontiguousarray(
        w.reshape(km, P, mm, P).transpose(2, 1, 0, 3)
    ).astype(bf16)


_NO_SPLIT = ("InstAllEngineBarrier", "InstCollectiveCompute",
             "InstEventSemaphore")


def _dedup_ldweights(nc):
    """Tile legalization emits one LDWEIGHTS per matmul; consecutive matmuls
    over the two batch halves share lhsT, so every second load is redundant
    (~108 ns x 2960 here). Drop an LDWEIGHTS when the previous PE instruction
    stream since the last LDWEIGHTS contains only matmuls and the weight AP
    is identical. Any waits it carried move to an EventSemaphore."""
    import concourse.mybir as mb
    ctr = 0
    for f in nc.m.functions:
        for b in f.blocks:
            out = []
            last_sig = None
            changed = False
            for ins in b.instructions:
                tn = type(ins).__name__
                if tn == "InstLdweights":
                    w = ins.ins[0]
                    sig = (str(w.memref), w.offset, str(w.ap), str(w.dtype),
                           str(ins.perf_mode), str(ins.is_transpose),
                           str(ins.tile_position))
                    if sig == last_sig:
                        si = getattr(ins, "sync_info", None)
                        waits = list(si.on_wait) if si is not None and si.on_wait else []
                        upds = list(si.on_update) if si is not None and si.on_update else []
                        for w_ in waits:
                            ev = mb.InstEventSemaphore(
                                name=f"ldwev-{ctr}", ins=[], outs=[])
                            ctr += 1
                            ev.engine = ins.engine
                            ev.sync_info = mb.SyncInfo(on_wait=[w_], on_update=[])
                            out.append(ev)
                        assert not upds, "dropped LDWEIGHTS carried an update"
                        changed = True
                        continue  # drop the redundant load
                    last_sig = sig
                elif getattr(ins, "engine", None) == mb.EngineType.PE \
                        and tn not in ("InstMatmult",):
                    last_sig = None  # anything else on PE invalidates reuse
                out.append(ins)
            if changed:
                b.instructions = out
    return nc


def _legalize_waits(nc, max_waits=1):
    """walrus codegen allows one sync-wait command per compute instruction;
    move extra waits onto preceding same-engine EventSemaphore carriers."""
    import concourse.mybir as mb
    ctr = 0
    for f in nc.m.functions:
        for b in f.blocks:
            il = b.instructions
            out = []
            changed = False
            for ins in il:
                si = getattr(ins, "sync_info", None)
                waits = list(si.on_wait) if si is not None and si.on_wait else []
                if len(waits) > max_waits and type(ins).__name__ not in _NO_SPLIT:
                    for w in waits[:-max_waits]:
                        ev = mb.InstEventSemaphore(
                            name=f"evsplit-{ctr}", ins=[], outs=[])
                        ctr += 1
                        ev.engine = ins.engine
                        ev.sync_info = mb.SyncInfo(on_wait=[w], on_update=[])
                        out.append(ev)
                    ins.sync_info = mb.SyncInfo(
                        on_wait=waits[-max_waits:], on_update=list(si.on_update))
                    changed = True
                out.append(ins)
            if changed:
                b.instructions = out
    return nc


def build_nc(NB, legalize=True):
    """Emit the Bass/Tile program for a per-core batch of NB rows."""
    NH = NB // 2
    nc = bass.Bass("TRN2", target_bir_lowering=False, debug=False,
                   num_devices=N_CORES)

    zt_d = nc.dram_tensor("zt", [P, KZ, NB], F32, kind="ExternalInput").ap()
    ct_d = nc.dram_tensor("ct", [KC, P, NB], BF16, kind="ExternalInput").ap()
    w1_d = nc.dram_tensor("w1", [L, MH, P, K1, P], BF16, kind="ExternalInput").ap()
    w2_d = nc.dram_tensor("w2", [L, MH, P, MH, P], BF16, kind="ExternalInput").ap()
    w3_d = nc.dram_tensor("w3", [L, MH, P, MH, P], BF16, kind="ExternalInput").ap()
    wst_d = nc.dram_tensor("wst", [L, MST, P, MH, P], BF16, kind="ExternalInput").ap()
    b123_d = nc.dram_tensor("b123", [P, 3, L, MH], F32, kind="ExternalInput").ap()
    bst_d = nc.dram_tensor("bst", [P, L, MST], F32, kind="ExternalInput").ap()
    oz_d = nc.dram_tensor("oz", [P, KZ, NB], F32, kind="ExternalOutput").ap()
    old_d = nc.dram_tensor("old", [1, NB], F32, kind="ExternalOutput").ap()

    with tile.TileContext(nc) as tc:
        with (
            tc.tile_pool(name="singles", bufs=1) as singles,
            tc.tile_pool(name="wpool", bufs=4) as wpool,
            tc.tile_pool(name="psum", bufs=4, space="PSUM") as psum,
            tc.tile_pool(name="ldps", bufs=2, space="PSUM") as ldps,
        ):
            # --- resident tensors ---
            # Prefetch layer-0's first weight tiles ahead of the big ctx DMA
            # so the PE can start within ~3 us instead of ~26 us.
            pre_w = {}
            for m in range(2):
                wt = wpool.tile([P, K1, P], BF16, tag=f"w{K1}", name=f"prew{m}")
                nc.sync.dma_start(out=wt[:], in_=w1_d[0, m])
                pre_w[m] = wt
            ct_sb = singles.tile([P, KC, NB], BF16, tag="ct")
            for k in range(KC):
                nc.sync.dma_start(out=ct_sb[:, k, :], in_=ct_d[k])
            z32 = singles.tile([P, KZ, NB], F32, tag="z32")
            nc.sync.dma_start(out=z32[:], in_=zt_d[:])
            zbf = singles.tile([P, KZ, NB], BF16, tag="zbf")
            nc.vector.tensor_copy(out=zbf[:], in_=z32[:])

            b123 = singles.tile([P, 3, L, MH], F32, tag="b123")
            nc.sync.dma_start(out=b123[:], in_=b123_d[:])
            bst_sb = singles.tile([P, L, MST], F32, tag="bst")
            nc.sync.dma_start(out=bst_sb[:], in_=bst_d[:])

            ones_sb = singles.tile([P, 1], BF16, tag="ones")
            nc.vector.memset(ones_sb, 1.0)
            ld_acc = singles.tile([1, NB], F32, tag="ldacc")
            nc.vector.memset(ld_acc, 0.0)

            h_sb = [singles.tile([P, MH, NB], BF16, tag=f"h{i}", name=f"h{i}")
                    for i in range(3)]
            st_tmp = singles.tile([P, 2, NB], F32, tag="sttmp")
            sc = singles.tile([P, 2, NB], BF16, tag="sc")
            ex = singles.tile([P, 2, NB], F32, tag="ex")
            sh = singles.tile([P, 2, NB], F32, tag="sh")

            def mm_layer(w_dram, l, km, mm, rhs_of_k, consume, preloaded=None):
                """out[m] = W[:, m].T @ act  for m in range(mm), K = km tiles."""
                for m in range(mm):
                    if preloaded and m in preloaded:
                        wt = preloaded[m]
                    else:
                        wt = wpool.tile([P, km, P], BF16, tag=f"w{km}")
                        nc.sync.dma_start(out=wt[:], in_=w_dram[l, m])
                    ps = [psum.tile([P, NH], F32, tag="ps", name=f"ps{i}")
                          for i in range(2)]
                    for k in range(km):
                        rhs = rhs_of_k(k)
                        for nh in range(2):
                            nc.tensor.matmul(
                                ps[nh], wt[:, k], rhs[:, nh * NH:(nh + 1) * NH],
                                start=(k == 0), stop=(k == km - 1),
                            )
                    for nh in range(2):
                        consume(m, nh, ps[nh])

            for l in range(L):
                def silu_to(dst, bvec):
                    def c(m, nh, ps):
                        nc.scalar.activation(
                            out=dst[:, m, nh * NH:(nh + 1) * NH], in_=ps,
                            func=AF.Silu, bias=bvec[:, l, m:m + 1], scale=1.0,
                        )
                    return c

                # mm1: backbone in = [ctx3 ; z]  (z tiles last so the PE can
                # start each layer before the previous z-update finishes)
                mm_layer(
                    w1_d, l, K1, MH,
                    lambda k: ct_sb[:, k] if k < KC else zbf[:, k - KC],
                    silu_to(h_sb[0], b123[:, 0]),
                    preloaded=pre_w if l == 0 else None,
                )
                mm_layer(w2_d, l, MH, MH, lambda k: h_sb[0][:, k],
                         silu_to(h_sb[1], b123[:, 1]))
                mm_layer(w3_d, l, MH, MH, lambda k: h_sb[1][:, k],
                         silu_to(h_sb[2], b123[:, 2]))

                # mm4: scale/shift head -> st_tmp (pre-clip scale) and sh
                def st_consume(m, nh, ps):
                    dst = st_tmp[:, m] if m < 2 else sh[:, m - 2]
                    nc.scalar.activation(
                        out=dst[:, nh * NH:(nh + 1) * NH], in_=ps,
                        func=AF.Identity, bias=bst_sb[:, l, m:m + 1], scale=1.0,
                    )
                mm_layer(wst_d, l, MH, MST, lambda k: h_sb[2][:, k], st_consume)

                # sc = clip(st_tmp) (bf16), ex = exp(sc)
                nc.vector.tensor_scalar(
                    out=sc[:], in0=st_tmp[:], scalar1=CLIP, scalar2=-CLIP,
                    op0=ALU.min, op1=ALU.max,
                )
                nc.scalar.activation(out=ex[:], in_=sc[:], func=AF.Exp)

                # logdet += ones.T @ sc  (sum over the 256 scale rows)
                for nh in range(2):
                    ldp = ldps.tile([1, NH], F32, tag="ld")
                    for k in range(KZ):
                        nc.tensor.matmul(
                            ldp, ones_sb, sc[:, k, nh * NH:(nh + 1) * NH],
                            start=(k == 0), stop=(k == KZ - 1),
                        )
                    nc.vector.tensor_add(
                        out=ld_acc[:, nh * NH:(nh + 1) * NH],
                        in0=ld_acc[:, nh * NH:(nh + 1) * NH], in1=ldp,
                    )

                # z <- z * ex + sh ; refresh bf16 copy for next layer's mm1
                nc.vector.tensor_mul(out=z32[:], in0=z32[:], in1=ex[:])
                nc.vector.tensor_add(out=z32[:], in0=z32[:], in1=sh[:])
                if l < L - 1:
                    nc.vector.tensor_copy(out=zbf[:], in_=z32[:])

            nc.sync.dma_start(out=oz_d[:], in_=z32[:])
            nc.sync.dma_start(out=old_d[:], in_=ld_acc[:])
    return _legalize_waits(_dedup_ldweights(nc)) if legalize else nc


def host_prep(inputs):
    """All input-independent restructuring. Returns (weight in_map, m_L)."""
    W1z, Wst, bst, m_L = _prep(
        np.asarray(inputs["perm"]), np.asarray(inputs["idx_a"]),
        np.asarray(inputs["idx_b"]), np.asarray(inputs["inv_pos"]),
        np.asarray(inputs["W1"], np.float32), np.asarray(inputs["Ws"], np.float32),
        np.asarray(inputs["bs"], np.float32), np.asarray(inputs["Wt"], np.float32),
        np.asarray(inputs["bt"], np.float32))
    W1 = np.asarray(inputs["W1"], np.float32)
    w1 = np.stack([
        _wtiles(np.concatenate([W1[l][A:], W1z[l]], axis=0), K1, MH)
        for l in range(L)
    ])
    w2 = np.stack([_wtiles(np.asarray(inputs["W2"][l], np.float32), MH, MH)
                   for l in range(L)])
    w3 = np.stack([_wtiles(np.asarray(inputs["W3"][l], np.float32), MH, MH)
                   for l in range(L)])
    wst = np.stack([_wtiles(Wst[l], MH, MST) for l in range(L)])
    b123 = np.ascontiguousarray(
        np.stack([np.asarray(inputs[n], np.float32) for n in ("b1", "b2", "b3")])
        .reshape(3, L, MH, P).transpose(3, 0, 1, 2))
    bsth = np.ascontiguousarray(bst.reshape(L, MST, P).transpose(2, 0, 1))
    wm = {"w1": w1, "w2": w2, "w3": w3, "wst": wst, "b123": b123, "bst": bsth}
    return wm, m_L


def core_inputs(inputs, NB=B // N_CORES):
    """Per-core activation shards (feature-major)."""
    x = np.asarray(inputs["x"], np.float32)
    c3 = np.concatenate([np.asarray(inputs["ctx"], np.float32),
                         np.asarray(inputs["t_emb"], np.float32),
                         np.asarray(inputs["cond_emb"], np.float32)], axis=1)
    outs = []
    for c in range(N_CORES):
        rows = slice(c * NB, (c + 1) * NB)
        zt = np.ascontiguousarray(
            x[rows].T.reshape(KZ, P, NB).transpose(1, 0, 2))
        ct = np.ascontiguousarray(c3[rows].T.astype(bf16)).reshape(KC, P, NB)
        outs.append({"zt": zt, "ct": ct})
    return outs


_NC_CACHE = {}


def kernel(**inputs):
    NB = B // N_CORES
    if NB not in _NC_CACHE:
        _NC_CACHE[NB] = build_nc(NB)
    nc = _NC_CACHE[NB]
    wm, m_L = host_prep(inputs)
    shards = core_inputs(inputs, NB)
    in_maps = [{**s, **wm} for s in shards]
    res = run_bass_kernel_spmd(nc, in_maps, list(range(N_CORES)))
    z_store = np.concatenate(
        [res.results[c]["oz"].transpose(1, 0, 2).reshape(D, NB).T
         for c in range(N_CORES)], axis=0)
    z = np.ascontiguousarray(z_store[:, np.argsort(m_L)], dtype=np.float32)
    logdet = np.concatenate(
        [res.results[c]["old"][0] for c in range(N_CORES)]).astype(np.float32)
    return z, logdet


# revision 27
# speedup vs baseline: 1.2157x; 1.0003x over previous
"""Trainium2 Bass kernel for an 8-layer ConditionalRealNVP flow.

Strategy
--------
Data-parallel: batch B=8192 is split across 8 NeuronCores (1024 rows each),
all parameters replicated. Activations live feature-major ([feature, batch])
in SBUF so every matmul is out[Mfeat, Nbatch] = W[K, M].T @ act[K, N] with
weights in natural [in, out] layout and outputs chaining directly into the
next matmul with no transposes.

All permutation / mask-split / scatter-merge index work is folded into the
weights on the host (input-independent):
  * z is kept in a fixed "storage order" on-device; a per-layer map
    storage-slot -> logical-feature is composed on the host.
  * x_a's contribution to the backbone input is W1z_eff = rows of W1[:A]
    scattered to the storage slots that hold x_a (other rows zero).
  * Ws/Wt columns are scattered to [H, 2D]: scale/shift for slot s land in
    columns s / D+s; x_a slots get zero columns (scale=0 -> exp=1, shift=0),
    so the affine update is uniformly  z <- z * exp(clip(scale)) + shift
    over all 256 slots, and logdet is a plain ones-vector matmul over the
    256 scale rows.
Matmul operands are bf16 (fp32 PSUM accumulation); z itself is carried in
fp32 across layers.
"""

import numpy as np
import ml_dtypes

import concourse.bass as bass
import concourse.mybir as mybir
import concourse.tile as tile
from concourse.bass_utils import run_bass_kernel_spmd

F32 = mybir.dt.float32
BF16 = mybir.dt.bfloat16
AF = mybir.ActivationFunctionType
ALU = mybir.AluOpType

L, D, H, B = 8, 256, 1024, 8192
CLIP = 3.0
A = D // 2
N_CORES = 8
P = 128
KC = 3 * H // P      # 24 ctx k-subtiles
KZ = D // P          # 2 z k-subtiles (emitted last in the mm1 K loop)
K1 = KC + KZ         # 26
MH = H // P          # 8 output tiles of the hidden layers
MST = 2 * D // P     # 4 output tiles of the scale/shift head
bf16 = ml_dtypes.bfloat16


def _prep(perm, idx_a, idx_b, inv_pos, W1, Ws, bs, Wt, bt):
    """Fold all index plumbing into weights; returns fp32 arrays + final map."""
    m = np.arange(D)
    W1z = np.zeros((L, D, H), np.float32)
    Wst = np.zeros((L, H, 2 * D), np.float32)
    bst = np.zeros((L, 2 * D), np.float32)
    for l in range(L):
        pos = np.empty(D, np.int64)
        pos[m] = np.arange(D)
        slots_a = pos[perm[l][idx_a[l]]]
        slots_b = pos[perm[l][idx_b[l]]]
        W1z[l][slots_a, :] = W1[l][:A, :]
        Wst[l][:, slots_b] = Ws[l]
        Wst[l][:, D + slots_b] = Wt[l]
        bst[l][slots_b] = bs[l]
        bst[l][D + slots_b] = bt[l]
        m_new = np.empty(D, np.int64)
        i = np.asarray(inv_pos[l])
        m_new[np.where(i < A, slots_a[np.minimum(i, A - 1)],
                       slots_b[np.maximum(i - A, 0)])] = np.arange(D)
        m = m_new
    return W1z, Wst, bst, m


def _wtiles(w, km, mm):
    """[K*P? , M*P] fp32 -> [mm, P, km, P] bf16 tiles, partition-contiguous."""
    k, mdim = w.shape
    assert k == km * P and mdim == mm * P
    return np.ascontiguousarray(
        w.reshape(km, P, mm, P).transpose(2, 1, 0, 3)
    ).astype(bf16)


_NO_SPLIT = ("InstAllEngineBarrier", "InstCollectiveCompute",
             "InstEventSemaphore")


def _dedup_ldweights(nc):
    """Tile legalization emits one LDWEIGHTS per matmul; consecutive matmuls
    over the two batch halves share lhsT, so every second load is redundant
    (~108 ns x 2960 here). Drop an LDWEIGHTS when the previous PE instruction
    stream since the last LDWEIGHTS contains only matmuls and the weight AP
    is identical. Any waits it carried move to an EventSemaphore."""
    import concourse.mybir as mb
    ctr = 0
    for f in nc.m.functions:
        for b in f.blocks:
            out = []
            last_sig = None
            changed = False
            for ins in b.instructions:
                tn = type(ins).__name__
                if tn == "InstLdweights":
                    w = ins.ins[0]
                    sig = (str(w.memref), w.offset, str(w.ap), str(w.dtype),
                           str(ins.perf_mode), str(ins.is_transpose),
                           str(ins.tile_position))
                    if sig == last_sig:
                        si = getattr(ins, "sync_info", None)
                        waits = list(si.on_wait) if si is not None and si.on_wait else []
                        upds = list(si.on_update) if si is not None and si.on_update else []
                        for w_ in waits:
                            ev = mb.InstEventSemaphore(
                                name=f"ldwev-{ctr}", ins=[], outs=[])
                            ctr += 1
                            ev.engine = ins.engine
                            ev.sync_info = mb.SyncInfo(on_wait=[w_], on_update=[])
                            out.append(ev)
                        assert not upds, "dropped LDWEIGHTS carried an update"
                        changed = True
                        continue  # drop the redundant load
                    last_sig = sig
                elif getattr(ins, "engine", None) == mb.EngineType.PE \
                        and tn not in ("InstMatmult",):
                    last_sig = None  # anything else on PE invalidates reuse
                out.append(ins)
            if changed:
                b.instructions = out
    return nc


def _legalize_waits(nc, max_waits=1):
    """walrus codegen allows one sync-wait command per compute instruction;
    move extra waits onto preceding same-engine EventSemaphore carriers."""
    import concourse.mybir as mb
    ctr = 0
    for f in nc.m.functions:
        for b in f.blocks:
            il = b.instructions
            out = []
            changed = False
            for ins in il:
                si = getattr(ins, "sync_info", None)
                waits = list(si.on_wait) if si is not None and si.on_wait else []
                if len(waits) > max_waits and type(ins).__name__ not in _NO_SPLIT:
                    for w in waits[:-max_waits]:
                        ev = mb.InstEventSemaphore(
                            name=f"evsplit-{ctr}", ins=[], outs=[])
                        ctr += 1
                        ev.engine = ins.engine
                        ev.sync_info = mb.SyncInfo(on_wait=[w], on_update=[])
                        out.append(ev)
                    ins.sync_info = mb.SyncInfo(
                        on_wait=waits[-max_waits:], on_update=list(si.on_update))
                    changed = True
                out.append(ins)
            if changed:
                b.instructions = out
    return nc


def build_nc(NB, legalize=True):
    """Emit the Bass/Tile program for a per-core batch of NB rows."""
    NH = NB // 2
    nc = bass.Bass("TRN2", target_bir_lowering=False, debug=False,
                   num_devices=N_CORES)

    zt_d = nc.dram_tensor("zt", [P, KZ, NB], F32, kind="ExternalInput").ap()
    ct_d = nc.dram_tensor("ct", [KC, P, NB], BF16, kind="ExternalInput").ap()
    w1_d = nc.dram_tensor("w1", [L, MH, P, K1, P], BF16, kind="ExternalInput").ap()
    w2_d = nc.dram_tensor("w2", [L, MH, P, MH, P], BF16, kind="ExternalInput").ap()
    w3_d = nc.dram_tensor("w3", [L, MH, P, MH, P], BF16, kind="ExternalInput").ap()
    wst_d = nc.dram_tensor("wst", [L, MST, P, MH, P], BF16, kind="ExternalInput").ap()
    b123_d = nc.dram_tensor("b123", [P, 3, L, MH], F32, kind="ExternalInput").ap()
    bst_d = nc.dram_tensor("bst", [P, L, MST], F32, kind="ExternalInput").ap()
    oz_d = nc.dram_tensor("oz", [P, KZ, NB], F32, kind="ExternalOutput").ap()
    old_d = nc.dram_tensor("old", [1, NB], F32, kind="ExternalOutput").ap()

    with tile.TileContext(nc) as tc:
        with (
            tc.tile_pool(name="singles", bufs=1) as singles,
            tc.tile_pool(name="wpool", bufs=4) as wpool,
            tc.tile_pool(name="psum", bufs=4, space="PSUM") as psum,
            tc.tile_pool(name="ldps", bufs=2, space="PSUM") as ldps,
        ):
            # --- resident tensors ---
            # Prefetch layer-0's first weight tiles ahead of the big ctx DMA
            # so the PE can start within ~3 us instead of ~26 us.
            pre_w = {}
            for m in range(2):
                wt = wpool.tile([P, K1, P], BF16, tag=f"w{K1}", name=f"prew{m}")
                nc.sync.dma_start(out=wt[:], in_=w1_d[0, m])
                pre_w[m] = wt
            ct_sb = singles.tile([P, KC, NB], BF16, tag="ct")
            for k in range(KC):
                nc.sync.dma_start(out=ct_sb[:, k, :], in_=ct_d[k])
            z32 = singles.tile([P, KZ, NB], F32, tag="z32")
            nc.sync.dma_start(out=z32[:], in_=zt_d[:])
            zbf = singles.tile([P, KZ, NB], BF16, tag="zbf")
            nc.vector.tensor_copy(out=zbf[:], in_=z32[:])

            b123 = singles.tile([P, 3, L, MH], F32, tag="b123")
            nc.sync.dma_start(out=b123[:], in_=b123_d[:])
            bst_sb = singles.tile([P, L, MST], F32, tag="bst")
            nc.sync.dma_start(out=bst_sb[:], in_=bst_d[:])

            ones_sb = singles.tile([P, 1], BF16, tag="ones")
            nc.vector.memset(ones_sb, 1.0)
            ld_acc = singles.tile([1, NB], F32, tag="ldacc")
            nc.vector.memset(ld_acc, 0.0)

            h_sb = [singles.tile([P, MH, NB], BF16, tag=f"h{i}", name=f"h{i}")
                    for i in range(3)]
            st_tmp = singles.tile([P, 2, NB], F32, tag="sttmp")
            sc = singles.tile([P, 2, NB], BF16, tag="sc")
            ex = singles.tile([P, 2, NB], F32, tag="ex")
            sh = singles.tile([P, 2, NB], F32, tag="sh")

            def mm_layer(w_dram, l, km, mm, rhs_of_k, consume, preloaded=None):
                """out[m] = W[:, m].T @ act  for m in range(mm), K = km tiles."""
                for m in range(mm):
                    if preloaded and m in preloaded:
                        wt = preloaded[m]
                    else:
                        wt = wpool.tile([P, km, P], BF16, tag=f"w{km}")
                        nc.sync.dma_start(out=wt[:], in_=w_dram[l, m])
                    ps = [psum.tile([P, NH], F32, tag="ps", name=f"ps{i}")
                          for i in range(2)]
                    for k in range(km):
                        rhs = rhs_of_k(k)
                        for nh in range(2):
                            nc.tensor.matmul(
                                ps[nh], wt[:, k], rhs[:, nh * NH:(nh + 1) * NH],
                                start=(k == 0), stop=(k == km - 1),
                            )
                    for nh in range(2):
                        consume(m, nh, ps[nh])

            for l in range(L):
                def silu_to(dst, bvec):
                    def c(m, nh, ps):
                        nc.scalar.activation(
                            out=dst[:, m, nh * NH:(nh + 1) * NH], in_=ps,
                            func=AF.Silu, bias=bvec[:, l, m:m + 1], scale=1.0,
                        )
                    return c

                # mm1: backbone in = [ctx3 ; z]  (z tiles last so the PE can
                # start each layer before the previous z-update finishes)
                mm_layer(
                    w1_d, l, K1, MH,
                    lambda k: ct_sb[:, k] if k < KC else zbf[:, k - KC],
                    silu_to(h_sb[0], b123[:, 0]),
                    preloaded=pre_w if l == 0 else None,
                )
                mm_layer(w2_d, l, MH, MH, lambda k: h_sb[0][:, k],
                         silu_to(h_sb[1], b123[:, 1]))
                mm_layer(w3_d, l, MH, MH, lambda k: h_sb[1][:, k],
                         silu_to(h_sb[2], b123[:, 2]))

                # mm4: scale/shift head -> st_tmp (pre-clip scale) and sh
                def st_consume(m, nh, ps):
                    dst = st_tmp[:, m] if m < 2 else sh[:, m - 2]
                    nc.scalar.activation(
                        out=dst[:, nh * NH:(nh + 1) * NH], in_=ps,
                        func=AF.Identity, bias=bst_sb[:, l, m:m + 1], scale=1.0,
                    )
                mm_layer(wst_d, l, MH, MST, lambda k: h_sb[2][:, k], st_consume)

                # sc = clip(st_tmp) (bf16), ex = exp(sc)
                nc.vector.tensor_scalar(
                    out=sc[:], in0=st_tmp[:], scalar1=CLIP, scalar2=-CLIP,
                    op0=ALU.min, op1=ALU.max,
                )
                nc.scalar.activation(out=ex[:], in_=sc[:], func=AF.Exp)

                # logdet += ones.T @ sc  (sum over the 256 scale rows)
                for nh in range(2):
                    ldp = ldps.tile([1, NH], F32, tag="ld")
                    for k in range(KZ):
                        nc.tensor.matmul(
                            ldp, ones_sb, sc[:, k, nh * NH:(nh + 1) * NH],
                            start=(k == 0), stop=(k == KZ - 1),
                        )
                    nc.vector.tensor_add(
                        out=ld_acc[:, nh * NH:(nh + 1) * NH],
                        in0=ld_acc[:, nh * NH:(nh + 1) * NH], in1=ldp,
                    )

                # z <- z * ex + sh ; refresh bf16 copy for next layer's mm1
                nc.vector.tensor_mul(out=z32[:], in0=z32[:], in1=ex[:])
                nc.vector.tensor_add(out=z32[:], in0=z32[:], in1=sh[:])
                if l < L - 1:
                    nc.vector.tensor_copy(out=zbf[:], in_=z32[:])

            nc.sync.dma_start(out=oz_d[:], in_=z32[:])
            nc.sync.dma_start(out=old_d[:], in_=ld_acc[:])
    return _legalize_waits(_dedup_ldweights(nc)) if legalize else nc


def host_prep(inputs):
    """All input-independent restructuring. Returns (weight in_map, m_L)."""
    W1z, Wst, bst, m_L = _prep(
        np.asarray(inputs["perm"]), np.asarray(inputs["idx_a"]),
        np.asarray(inputs["idx_b"]), np.asarray(inputs["inv_pos"]),
        np.asarray(inputs["W1"], np.float32), np.asarray(inputs["Ws"], np.float32),
        np.asarray(inputs["bs"], np.float32), np.asarray(inputs["Wt"], np.float32),
        np.asarray(inputs["bt"], np.float32))
    W1 = np.asarray(inputs["W1"], np.float32)
    w1 = np.stack([
        _wtiles(np.concatenate([W1[l][A:], W1z[l]], axis=0), K1, MH)
        for l in range(L)
    ])
    w2 = np.stack([_wtiles(np.asarray(inputs["W2"][l], np.float32), MH, MH)
                   for l in range(L)])
    w3 = np.stack([_wtiles(np.asarray(inputs["W3"][l], np.float32), MH, MH)
                   for l in range(L)])
    wst = np.stack([_wtiles(Wst[l], MH, MST) for l in range(L)])
    b123 = np.ascontiguousarray(
        np.stack([np.asarray(inputs[n], np.float32) for n in ("b1", "b2", "b3")])
        .reshape(3, L, MH, P).transpose(3, 0, 1, 2))
    bsth = np.ascontiguousarray(bst.reshape(L, MST, P).transpose(2, 0, 1))
    wm = {"w1": w1, "w2": w2, "w3": w3, "wst": wst, "b123": b123, "bst": bsth}
    return wm, m_L


def core_inputs(inputs, NB=B // N_CORES):
    """Per-core activation shards (feature-major)."""
    x = np.asarray(inputs["x"], np.float32)
    c3 = np.concatenate([np.asarray(inputs["ctx"], np.float32),
                         np.asarray(inputs["t_emb"], np.float32),
                         np.asarray(inputs["cond_emb"], np.float32)], axis=1)
    outs = []
    for c in range(N_CORES):
        rows = slice(c * NB, (c + 1) * NB)
        zt = np.ascontiguousarray(
            x[rows].T.reshape(KZ, P, NB).transpose(1, 0, 2))
        ct = np.ascontiguousarray(c3[rows].T.astype(bf16)).reshape(KC, P, NB)
        outs.append({"zt": zt, "ct": ct})
    return outs


_NC_CACHE = {}


def kernel(**inputs):
    NB = B // N_CORES
    if NB not in _NC_CACHE:
        _NC_CACHE[NB] = build_nc(NB)
    nc = _NC_CACHE[NB]
    wm, m_L = host_prep(inputs)
    shards = core_inputs(inputs, NB)
    in_maps = [{**s, **wm} for s in shards]
    res = run_bass_kernel_spmd(nc, in_maps, list(range(N_CORES)))
    z_store = np.concatenate(
        [res.results[c]["oz"].transpose(1, 0, 2).reshape(D, NB).T
         for c in range(N_CORES)], axis=0)
    z = np.ascontiguousarray(z_store[:, np.argsort(m_L)], dtype=np.float32)
    logdet = np.concatenate(
        [res.results[c]["old"][0] for c in range(N_CORES)]).astype(np.float32)
    return z, logdet


# revision 33
# speedup vs baseline: 1.2166x; 1.0007x over previous
"""Trainium2 Bass kernel for an 8-layer ConditionalRealNVP flow.

Strategy
--------
Data-parallel: batch B=8192 is split across 8 NeuronCores (1024 rows each),
all parameters replicated. Activations live feature-major ([feature, batch])
in SBUF so every matmul is out[Mfeat, Nbatch] = W[K, M].T @ act[K, N] with
weights in natural [in, out] layout and outputs chaining directly into the
next matmul with no transposes.

All permutation / mask-split / scatter-merge index work is folded into the
weights on the host (input-independent):
  * z is kept in a fixed "storage order" on-device; a per-layer map
    storage-slot -> logical-feature is composed on the host.
  * x_a's contribution to the backbone input is W1z_eff = rows of W1[:A]
    scattered to the storage slots that hold x_a (other rows zero).
  * Ws/Wt columns are scattered to [H, 2D]: scale/shift for slot s land in
    columns s / D+s; x_a slots get zero columns (scale=0 -> exp=1, shift=0),
    so the affine update is uniformly  z <- z * exp(clip(scale)) + shift
    over all 256 slots, and logdet is a plain ones-vector matmul over the
    256 scale rows.
Matmul operands are bf16 (fp32 PSUM accumulation); z itself is carried in
fp32 across layers.
"""

import numpy as np
import ml_dtypes

import concourse.bass as bass
import concourse.mybir as mybir
import concourse.tile as tile
from concourse.bass_utils import run_bass_kernel_spmd

F32 = mybir.dt.float32
BF16 = mybir.dt.bfloat16
AF = mybir.ActivationFunctionType
ALU = mybir.AluOpType

L, D, H, B = 8, 256, 1024, 8192
CLIP = 3.0
A = D // 2
N_CORES = 8
P = 128
KC = 3 * H // P      # 24 ctx k-subtiles
KZ = D // P          # 2 z k-subtiles (emitted last in the mm1 K loop)
K1 = KC + KZ         # 26
MH = H // P          # 8 output tiles of the hidden layers
MST = 2 * D // P     # 4 output tiles of the scale/shift head
bf16 = ml_dtypes.bfloat16


def _prep(perm, idx_a, idx_b, inv_pos, W1, Ws, bs, Wt, bt):
    """Fold all index plumbing into weights; returns fp32 arrays + final map."""
    m = np.arange(D)
    W1z = np.zeros((L, D, H), np.float32)
    Wst = np.zeros((L, H, 2 * D), np.float32)
    bst = np.zeros((L, 2 * D), np.float32)
    for l in range(L):
        pos = np.empty(D, np.int64)
        pos[m] = np.arange(D)
        slots_a = pos[perm[l][idx_a[l]]]
        slots_b = pos[perm[l][idx_b[l]]]
        W1z[l][slots_a, :] = W1[l][:A, :]
        Wst[l][:, slots_b] = Ws[l]
        Wst[l][:, D + slots_b] = Wt[l]
        bst[l][slots_b] = bs[l]
        bst[l][D + slots_b] = bt[l]
        m_new = np.empty(D, np.int64)
        i = np.asarray(inv_pos[l])
        m_new[np.where(i < A, slots_a[np.minimum(i, A - 1)],
                       slots_b[np.maximum(i - A, 0)])] = np.arange(D)
        m = m_new
    return W1z, Wst, bst, m


def _wtiles(w, km, mm):
    """[K*P? , M*P] fp32 -> [mm, P, km, P] bf16 tiles, partition-contiguous."""
    k, mdim = w.shape
    assert k == km * P and mdim == mm * P
    return np.ascontiguousarray(
        w.reshape(km, P, mm, P).transpose(2, 1, 0, 3)
    ).astype(bf16)


_NO_SPLIT = ("InstAllEngineBarrier", "InstCollectiveCompute",
             "InstEventSemaphore")


def _dedup_ldweights(nc):
    """Tile legalization emits one LDWEIGHTS per matmul; consecutive matmuls
    over the two batch halves share lhsT, so every second load is redundant
    (~108 ns x 2960 here). Drop an LDWEIGHTS when the previous PE instruction
    stream since the last LDWEIGHTS contains only matmuls and the weight AP
    is identical. Any waits it carried move to an EventSemaphore."""
    import concourse.mybir as mb
    ctr = 0
    for f in nc.m.functions:
        for b in f.blocks:
            out = []
            last_sig = None
            changed = False
            for ins in b.instructions:
                tn = type(ins).__name__
                if tn == "InstLdweights":
                    w = ins.ins[0]
                    sig = (str(w.memref), w.offset, str(w.ap), str(w.dtype),
                           str(ins.perf_mode), str(ins.is_transpose),
                           str(ins.tile_position))
                    if sig == last_sig:
                        si = getattr(ins, "sync_info", None)
                        waits = list(si.on_wait) if si is not None and si.on_wait else []
                        upds = list(si.on_update) if si is not None and si.on_update else []
                        for w_ in waits:
                            ev = mb.InstEventSemaphore(
                                name=f"ldwev-{ctr}", ins=[], outs=[])
                            ctr += 1
                            ev.engine = ins.engine
                            ev.sync_info = mb.SyncInfo(on_wait=[w_], on_update=[])
                            out.append(ev)
                        assert not upds, "dropped LDWEIGHTS carried an update"
                        changed = True
                        continue  # drop the redundant load
                    last_sig = sig
                elif getattr(ins, "engine", None) == mb.EngineType.PE \
                        and tn not in ("InstMatmult",):
                    last_sig = None  # anything else on PE invalidates reuse
                out.append(ins)
            if changed:
                b.instructions = out
    return nc


def _legalize_waits(nc, max_waits=1):
    """walrus codegen allows one sync-wait command per compute instruction;
    move extra waits onto preceding same-engine EventSemaphore carriers."""
    import concourse.mybir as mb
    ctr = 0
    for f in nc.m.functions:
        for b in f.blocks:
            il = b.instructions
            out = []
            changed = False
            for ins in il:
                si = getattr(ins, "sync_info", None)
                waits = list(si.on_wait) if si is not None and si.on_wait else []
                if len(waits) > max_waits and type(ins).__name__ not in _NO_SPLIT:
                    for w in waits[:-max_waits]:
                        ev = mb.InstEventSemaphore(
                            name=f"evsplit-{ctr}", ins=[], outs=[])
                        ctr += 1
                        ev.engine = ins.engine
                        ev.sync_info = mb.SyncInfo(on_wait=[w], on_update=[])
                        out.append(ev)
                    ins.sync_info = mb.SyncInfo(
                        on_wait=waits[-max_waits:], on_update=list(si.on_update))
                    changed = True
                out.append(ins)
            if changed:
                b.instructions = out
    return nc


def build_nc(NB, legalize=True):
    """Emit the Bass/Tile program for a per-core batch of NB rows."""
    NH = NB // 2
    nc = bass.Bass("TRN2", target_bir_lowering=False, debug=False,
                   num_devices=N_CORES)

    zt_d = nc.dram_tensor("zt", [P, KZ, NB], F32, kind="ExternalInput").ap()
    ct_d = nc.dram_tensor("ct", [KC, P, NB], BF16, kind="ExternalInput").ap()
    w1_d = nc.dram_tensor("w1", [L, MH, P, K1, P], BF16, kind="ExternalInput").ap()
    w2_d = nc.dram_tensor("w2", [L, MH, P, MH, P], BF16, kind="ExternalInput").ap()
    w3_d = nc.dram_tensor("w3", [L, MH, P, MH, P], BF16, kind="ExternalInput").ap()
    wst_d = nc.dram_tensor("wst", [L, MST, P, MH, P], BF16, kind="ExternalInput").ap()
    b123_d = nc.dram_tensor("b123", [P, 3, L, MH], F32, kind="ExternalInput").ap()
    bst_d = nc.dram_tensor("bst", [P, L, MST], F32, kind="ExternalInput").ap()
    oz_d = nc.dram_tensor("oz", [P, KZ, NB], F32, kind="ExternalOutput").ap()
    old_d = nc.dram_tensor("old", [1, NB], F32, kind="ExternalOutput").ap()

    with tile.TileContext(nc) as tc:
        with (
            tc.tile_pool(name="singles", bufs=1) as singles,
            tc.tile_pool(name="wpool", bufs=4) as wpool,
            tc.tile_pool(name="psum", bufs=4, space="PSUM") as psum,
            tc.tile_pool(name="ldps", bufs=2, space="PSUM") as ldps,
        ):
            # --- resident tensors ---
            # Prefetch layer-0's first weight tiles ahead of the big ctx DMA
            # so the PE can start within ~3 us instead of ~26 us.
            pre_w = {}
            for m in range(2):
                wt = wpool.tile([P, K1, P], BF16, tag=f"w{K1}", name=f"prew{m}")
                nc.sync.dma_start(out=wt[:], in_=w1_d[0, m])
                pre_w[m] = wt
            ct_sb = singles.tile([P, KC, NB], BF16, tag="ct")
            for k in range(KC):
                nc.sync.dma_start(out=ct_sb[:, k, :], in_=ct_d[k])
            z32 = singles.tile([P, KZ, NB], F32, tag="z32")
            nc.sync.dma_start(out=z32[:], in_=zt_d[:])
            zbf = singles.tile([P, KZ, NB], BF16, tag="zbf")
            nc.vector.tensor_copy(out=zbf[:], in_=z32[:])

            b123 = singles.tile([P, 3, L, MH], F32, tag="b123")
            nc.sync.dma_start(out=b123[:], in_=b123_d[:])
            bst_sb = singles.tile([P, L, MST], F32, tag="bst")
            nc.sync.dma_start(out=bst_sb[:], in_=bst_d[:])

            ones_sb = singles.tile([P, 1], BF16, tag="ones")
            nc.vector.memset(ones_sb, 1.0)
            ld_acc = singles.tile([1, NB], F32, tag="ldacc")
            nc.vector.memset(ld_acc, 0.0)

            h_sb = [singles.tile([P, MH, NB], BF16, tag=f"h{i}", name=f"h{i}")
                    for i in range(3)]
            st_tmp = singles.tile([P, 2, NB], F32, tag="sttmp")
            sc = singles.tile([P, 2, NB], BF16, tag="sc")
            ex = singles.tile([P, 2, NB], F32, tag="ex")
            sh = singles.tile([P, 2, NB], F32, tag="sh")

            def mm_layer(w_dram, l, km, mm, rhs_of_k, consume, preloaded=None):
                """out[m] = W[:, m].T @ act  for m in range(mm), K = km tiles."""
                for m in range(mm):
                    if preloaded and m in preloaded:
                        wt = preloaded[m]
                    else:
                        wt = wpool.tile([P, km, P], BF16, tag=f"w{km}")
                        nc.sync.dma_start(out=wt[:], in_=w_dram[l, m])
                    ps = [psum.tile([P, NH], F32, tag="ps", name=f"ps{i}")
                          for i in range(2)]
                    for k in range(km):
                        rhs = rhs_of_k(k)
                        for nh in range(2):
                            nc.tensor.matmul(
                                ps[nh], wt[:, k], rhs[:, nh * NH:(nh + 1) * NH],
                                start=(k == 0), stop=(k == km - 1),
                            )
                    for nh in range(2):
                        consume(m, nh, ps[nh])

            for l in range(L):
                def silu_to(dst, bvec):
                    def c(m, nh, ps):
                        nc.scalar.activation(
                            out=dst[:, m, nh * NH:(nh + 1) * NH], in_=ps,
                            func=AF.Silu, bias=bvec[:, l, m:m + 1], scale=1.0,
                        )
                    return c

                # mm1: backbone in = [ctx3 ; z]  (z tiles last so the PE can
                # start each layer before the previous z-update finishes)
                mm_layer(
                    w1_d, l, K1, MH,
                    lambda k: ct_sb[:, k] if k < KC else zbf[:, k - KC],
                    silu_to(h_sb[0], b123[:, 0]),
                    preloaded=pre_w if l == 0 else None,
                )
                mm_layer(w2_d, l, MH, MH, lambda k: h_sb[0][:, k],
                         silu_to(h_sb[1], b123[:, 1]))
                mm_layer(w3_d, l, MH, MH, lambda k: h_sb[1][:, k],
                         silu_to(h_sb[2], b123[:, 2]))

                # mm4: scale/shift head -> st_tmp (pre-clip scale) and sh
                def st_consume(m, nh, ps):
                    dst = st_tmp[:, m] if m < 2 else sh[:, m - 2]
                    nc.scalar.activation(
                        out=dst[:, nh * NH:(nh + 1) * NH], in_=ps,
                        func=AF.Identity, bias=bst_sb[:, l, m:m + 1], scale=1.0,
                    )
                mm_layer(wst_d, l, MH, MST, lambda k: h_sb[2][:, k], st_consume)

                # sc = clip(st_tmp) (bf16), ex = exp(sc)
                nc.vector.tensor_scalar(
                    out=sc[:], in0=st_tmp[:], scalar1=CLIP, scalar2=-CLIP,
                    op0=ALU.min, op1=ALU.max,
                )
                nc.scalar.activation(out=ex[:], in_=sc[:], func=AF.Exp)

                # logdet += ones.T @ sc  (sum over the 256 scale rows)
                for nh in range(2):
                    ldp = ldps.tile([1, NH], F32, tag="ld")
                    for k in range(KZ):
                        nc.tensor.matmul(
                            ldp, ones_sb, sc[:, k, nh * NH:(nh + 1) * NH],
                            start=(k == 0), stop=(k == KZ - 1),
                        )
                    nc.vector.tensor_add(
                        out=ld_acc[:, nh * NH:(nh + 1) * NH],
                        in0=ld_acc[:, nh * NH:(nh + 1) * NH], in1=ldp,
                    )

                # z <- z * ex + sh ; refresh bf16 copy for next layer's mm1
                nc.vector.tensor_mul(out=z32[:], in0=z32[:], in1=ex[:])
                nc.vector.tensor_add(out=z32[:], in0=z32[:], in1=sh[:])
                if l < L - 1:
                    nc.vector.tensor_copy(out=zbf[:], in_=z32[:])

            nc.sync.dma_start(out=oz_d[:], in_=z32[:])
            nc.sync.dma_start(out=old_d[:], in_=ld_acc[:])
    return _legalize_waits(_dedup_ldweights(nc)) if legalize else nc


def host_prep(inputs):
    """All input-independent restructuring. Returns (weight in_map, m_L)."""
    W1z, Wst, bst, m_L = _prep(
        np.asarray(inputs["perm"]), np.asarray(inputs["idx_a"]),
        np.asarray(inputs["idx_b"]), np.asarray(inputs["inv_pos"]),
        np.asarray(inputs["W1"], np.float32), np.asarray(inputs["Ws"], np.float32),
        np.asarray(inputs["bs"], np.float32), np.asarray(inputs["Wt"], np.float32),
        np.asarray(inputs["bt"], np.float32))
    W1 = np.asarray(inputs["W1"], np.float32)
    w1 = np.stack([
        _wtiles(np.concatenate([W1[l][A:], W1z[l]], axis=0), K1, MH)
        for l in range(L)
    ])
    w2 = np.stack([_wtiles(np.asarray(inputs["W2"][l], np.float32), MH, MH)
                   for l in range(L)])
    w3 = np.stack([_wtiles(np.asarray(inputs["W3"][l], np.float32), MH, MH)
                   for l in range(L)])
    wst = np.stack([_wtiles(Wst[l], MH, MST) for l in range(L)])
    b123 = np.ascontiguousarray(
        np.stack([np.asarray(inputs[n], np.float32) for n in ("b1", "b2", "b3")])
        .reshape(3, L, MH, P).transpose(3, 0, 1, 2))
    bsth = np.ascontiguousarray(bst.reshape(L, MST, P).transpose(2, 0, 1))
    wm = {"w1": w1, "w2": w2, "w3": w3, "wst": wst, "b123": b123, "bst": bsth}
    return wm, m_L


def core_inputs(inputs, NB=B // N_CORES):
    """Per-core activation shards (feature-major)."""
    x = np.asarray(inputs["x"], np.float32)
    c3 = np.concatenate([np.asarray(inputs["ctx"], np.float32),
                         np.asarray(inputs["t_emb"], np.float32),
                         np.asarray(inputs["cond_emb"], np.float32)], axis=1)
    outs = []
    for c in range(N_CORES):
        rows = slice(c * NB, (c + 1) * NB)
        zt = np.ascontiguousarray(
            x[rows].T.reshape(KZ, P, NB).transpose(1, 0, 2))
        ct = np.ascontiguousarray(c3[rows].T.astype(bf16)).reshape(KC, P, NB)
        outs.append({"zt": zt, "ct": ct})
    return outs


_NC_CACHE = {}


def kernel(**inputs):
    NB = B // N_CORES
    if NB not in _NC_CACHE:
        _NC_CACHE[NB] = build_nc(NB)
    nc = _NC_CACHE[NB]
    wm, m_L = host_prep(inputs)
    shards = core_inputs(inputs, NB)
    in_maps = [{**s, **wm} for s in shards]
    res = run_bass_kernel_spmd(nc, in_maps, list(range(N_CORES)))
    z_store = np.concatenate(
        [res.results[c]["oz"].transpose(1, 0, 2).reshape(D, NB).T
         for c in range(N_CORES)], axis=0)
    z = np.ascontiguousarray(z_store[:, np.argsort(m_L)], dtype=np.float32)
    logdet = np.concatenate(
        [res.results[c]["old"][0] for c in range(N_CORES)]).astype(np.float32)
    return z, logdet
